# revision 32
# baseline (speedup 1.0000x reference)
"""Trainium2 Bass kernel for nn_DNF_21071109554827 (soft DNF rule network).

Math: the conjunct product prod_k(x_k*a0 + (1-x_k)*a1 + a2) over NUM_IN=160
inputs is separable in log space into 5 blocks (nullary / unary slot a /
unary slot b / binary (a,b) / binary (b,a)).  The per-block tables are
computed over the full 16x16 object grid so every gather is an affine
access pattern; diagonal (invalid) entries are dropped on the host.

Sharding: the 96 (rule r, conjunct c) pairs are split across 8 cores
(3 rules x 4 conjuncts each).  Each core computes partial
sum_{c in core} ln(1 - ok*conj) -> [3, B*16*16]; the host adds partials
across cores (= product over all 32 conjuncts), finishes the
probabilistic sums, and returns full outputs.

Device layout per core:
  partition = (rc_local//...) packed as 4 rc x 32 k = 128 lanes
  ACT computes Ln(x*d + e) with per-partition scale/bias (d, e) --
  one fused instruction per (table, rc-group, column-split).
  PE reduces over k (block-diag ones lhsT) and accumulates all 5 tables
  straight into U[12, 512] PSUM chunks, using broadcast access patterns
  to expand the small tables over the (a, bb) grid.
  ACT: conj = Exp(U); DVE: u = 1 - ok*conj (exact fp32) -> DMA out.
  Host multiplies u across all 32 conjuncts (fp32, matching the
  reference's product semantics bit-for-bit in the common tiny-t case).
"""

import numpy as np

# ---------------- static config (hardcoded; must match the problem) -------
B, O, V = 16, 16, 2
P0 = P1 = P2 = 32
R, C = 3, 32
NPERM = O * (O - 1)
NUM_IN = P0 + V * P1 + V * (V - 1) * P2  # 160
N_CORES = 8
CPC = C // N_CORES        # conjuncts per core = 4
GRID = B * O * O          # 4096 free columns (b, a, bb)
NSPLIT = 2                # column splits for ACT/DMA pipelining
HALF = GRID // NSPLIT
NCHUNK = GRID // 512      # 8 PSUM chunks of 512 cols

_cache = {}


def _split_waits(nc, cap=1):
    """The container's walrus rejects instructions with more than ~1 sync
    wait.  Move excess waits onto injected same-engine NOPs placed right
    before the instruction (same serial engine stream -> semantics kept)."""
    import concourse.mybir as mb

    uid = [0]
    for f in nc.m.functions:
        for blk in f.blocks:
            out = []
            changed = False
            for inst in blk.instructions:
                si = inst.sync_info
                waits = list(si.on_wait) if si is not None else []
                if len(waits) > cap:
                    changed = True
                    for w in waits[:-cap]:
                        nop = mb.InstNoOp(
                            name=f"waitsplit-{uid[0]}", ins=[], outs=[])
                        uid[0] += 1
                        nop.engine = inst.engine
                        nop.sync_info = mb.SyncInfo(on_wait=[w], on_update=[])
                        out.append(nop)
                    inst.sync_info = mb.SyncInfo(
                        on_wait=waits[-cap:], on_update=list(si.on_update))
                out.append(inst)
            if changed:
                blk.instructions = out


def _build_bass(debug=False, split=True):
    import concourse.bass as bass
    import concourse.mybir as mybir
    from concourse import tile

    fp32 = mybir.dt.float32
    AF = mybir.ActivationFunctionType

    nc = bass.Bass()
    if debug:
        dbg_conj = nc.dram_tensor("dbg_conj", [12, GRID], fp32,
                                  kind="ExternalOutput")
        dbg_l3 = nc.dram_tensor("dbg_l3", [128, GRID], fp32,
                                kind="ExternalOutput")
    xb = nc.dram_tensor("xb", [128, GRID], fp32, kind="ExternalInput")
    xbT = nc.dram_tensor("xbT", [128, GRID], fp32, kind="ExternalInput")
    xu = nc.dram_tensor("xu", [128, B * O], fp32, kind="ExternalInput")
    xn = nc.dram_tensor("xn", [128, B], fp32, kind="ExternalInput")
    scl = nc.dram_tensor("scl", [128, 15], fp32, kind="ExternalInput")
    bia = nc.dram_tensor("bia", [128, 15], fp32, kind="ExternalInput")
    lnok = nc.dram_tensor("lnok", [12, 1], fp32, kind="ExternalInput")
    bdu = nc.dram_tensor("bdu", [128, 36], fp32, kind="ExternalInput")
    uo = nc.dram_tensor("uo", [12, GRID], fp32, kind="ExternalOutput")

    with tile.TileContext(nc) as tc:
        with (
            tc.tile_pool(name="const", bufs=1) as cpool,
            tc.tile_pool(name="big", bufs=1) as bpool,
            tc.tile_pool(name="psumU", bufs=6, space="PSUM") as ppool,
            tc.tile_pool(name="psumC", bufs=1, space="PSUM") as qpool,
            tc.tile_pool(name="tail", bufs=4) as tpool,
        ):
            # ---- load constants / inputs ----
            s_xu = cpool.tile([128, B * O], fp32, tag="xu")
            s_xn = cpool.tile([128, B], fp32, tag="xn")
            s_scl = cpool.tile([128, 15], fp32, tag="scl")
            s_bia = cpool.tile([128, 15], fp32, tag="bia")
            s_lnok = cpool.tile([12, 1], fp32, tag="lnok")
            s_bdu = cpool.tile([128, 36], fp32, tag="bdu")
            nc.scalar.dma_start(s_scl[:], scl[:])
            nc.scalar.dma_start(s_bia[:], bia[:])
            nc.gpsimd.dma_start(s_xu[:], xu[:])
            nc.gpsimd.dma_start(s_xn[:], xn[:])
            nc.gpsimd.dma_start(s_lnok[:], lnok[:])
            nc.gpsimd.dma_start(s_bdu[:], bdu[:])
            # warm the ACT Ln/Exp PWP tables while DMAs stream
            warm = cpool.tile([1, 1], fp32, tag="warm", name="warm")
            nc.gpsimd.memset(warm[:], 1.0)
            nc.scalar.activation(warm[:], warm[:], AF.Ln)
            nc.scalar.activation(warm[:], warm[:], AF.Exp)

            # ---- big Ln tables (flat grids; xbT pre-transposed on host) ----
            # pass t: 0=S3'(k96), 1=S4'(k128), 2=S1(k32), 3=S2(k64), 4=S0(k0)
            s_xb = bpool.tile([128, GRID], fp32, tag="xb")
            s_xbT = bpool.tile([128, GRID], fp32, tag="xbT")
            L3 = [[None] * 3 for _ in range(NSPLIT)]
            L4 = [[None] * 3 for _ in range(NSPLIT)]
            compact_done = False
            for h in range(NSPLIT):
                sl = slice(h * HALF, (h + 1) * HALF)
                nc.sync.dma_start(s_xb[:, sl], xb[:, sl])
                nc.gpsimd.dma_start(s_xbT[:, sl], xbT[:, sl])
                for g in range(3):
                    t3 = bpool.tile([128, HALF], fp32, tag=f"L3_{g}_{h}")
                    t4 = bpool.tile([128, HALF], fp32, tag=f"L4_{g}_{h}")
                    j0 = g * 5 + 0
                    j1 = g * 5 + 1
                    nc.scalar.activation(
                        t3[:], s_xb[:, sl], AF.Ln,
                        bias=s_bia[:, j0:j0 + 1], scale=s_scl[:, j0:j0 + 1])
                    nc.scalar.activation(
                        t4[:], s_xbT[:, sl], AF.Ln,
                        bias=s_bia[:, j1:j1 + 1], scale=s_scl[:, j1:j1 + 1])
                    L3[h][g] = t3
                    L4[h][g] = t4
                if compact_done:
                    continue
                compact_done = True
                # ---- compact tables S1/S2/S0 -> E1 = exp(S0+S1+ln ok), E2
                u1 = qpool.tile([12, B * O], fp32, tag="U1")
                u2 = qpool.tile([12, B * O], fp32, tag="U2")
                for g in range(3):
                    t1 = cpool.tile([128, B * O], fp32, tag=f"L1_{g}")
                    t2 = cpool.tile([128, B * O], fp32, tag=f"L2_{g}")
                    t0 = cpool.tile([128, B], fp32, tag=f"L0_{g}")
                    for t, (tt, xin) in enumerate(
                            [(t1, s_xu), (t2, s_xu), (t0, s_xn)], start=2):
                        j = g * 5 + t
                        nc.scalar.activation(
                            tt[:], xin[:], AF.Ln,
                            bias=s_bia[:, j:j + 1], scale=s_scl[:, j:j + 1])
                    w_g = s_bdu[:, 12 * g:12 * g + 12]
                    nc.tensor.matmul(u1[:], w_g, t1[:],
                                     start=(g == 0), stop=False)
                    r0 = t0[:, :, None].broadcast_to((128, B, O))
                    nc.tensor.matmul(
                        u1[:].rearrange("p (b o) -> p b o", b=B), w_g, r0,
                        start=False, stop=(g == 2))
                    nc.tensor.matmul(u2[:], w_g, t2[:],
                                     start=(g == 0), stop=(g == 2))
                e1 = cpool.tile([12, B * O], fp32, tag="E1")
                e2 = cpool.tile([12, B * O], fp32, tag="E2")
                nc.scalar.activation(e1[:], u1[:], AF.Exp, bias=s_lnok[:])
                nc.scalar.activation(e2[:], u2[:], AF.Exp)

            # ---- U accumulation: g-outer so PE keeps each weight loaded ----
            U = [None] * NCHUNK
            for h in range(NSPLIT):
                for g in range(3):
                    w_g = s_bdu[:, 12 * g:12 * g + 12]
                    for m in range(h * NCHUNK // NSPLIT,
                                   (h + 1) * NCHUNK // NSPLIT):
                        off = m * 512 - h * HALF
                        if g == 0:
                            U[m] = ppool.tile([12, 512], fp32, tag="U", name=f"U{m}")
                        nc.tensor.matmul(
                            U[m][:], w_g, L3[h][g][:, off:off + 512],
                            start=(g == 0), stop=False)
                        nc.tensor.matmul(
                            U[m][:], w_g, L4[h][g][:, off:off + 512],
                            start=False, stop=(g == 2))
                # ---- tails for this half ----
                for m in range(h * NCHUNK // NSPLIT,
                               (h + 1) * NCHUNK // NSPLIT):
                    bpair = slice(2 * m, 2 * m + 2)
                    nm = tpool.tile([12, 512], fp32, tag="nm")
                    r1 = e1[:].rearrange("p (b o) -> p b o", b=B)
                    r1 = r1[:, bpair, :, None].broadcast_to((12, 2, O, O))
                    r2 = e2[:].rearrange("p (b o) -> p b o", b=B)
                    r2 = r2[:, bpair, None, :].broadcast_to((12, 2, O, O))
                    nc.vector.tensor_tensor(
                        nm[:].rearrange("p (b x y) -> p b x y", b=2, x=O),
                        r1, r2, mybir.AluOpType.mult)
                    e34 = tpool.tile([12, 512], fp32, tag="e34")
                    nc.scalar.activation(e34[:], U[m][:], AF.Exp)
                    w = tpool.tile([12, 512], fp32, tag="w")
                    nc.vector.scalar_tensor_tensor(
                        w[:], e34[:], -1.0, nm[:],
                        mybir.AluOpType.mult, mybir.AluOpType.mult)
                    nc.sync.dma_start(uo[:, m * 512:(m + 1) * 512], w[:])

    if split:
        _split_waits(nc)
    return nc


def _host_prep(nullary, unary, binary, and_kernel, or_kernel, temperature):
    """Everything cheap: softmax/sigmoid, tables, per-core input maps."""
    t = np.float64(temperature.reshape(-1)[0])
    akd = and_kernel.astype(np.float64) / t
    akd = akd - akd.max(axis=-1, keepdims=True)
    eak = np.exp(akd)
    ak = (eak / eak.sum(axis=-1, keepdims=True))          # [R,C,160,3] f64
    ok = 1.0 / (1.0 + np.exp(-or_kernel.astype(np.float64) / t))  # [R,C]

    d = (ak[..., 0] - ak[..., 1]).astype(np.float32)      # [R,C,160]
    e = (ak[..., 1] + ak[..., 2]).astype(np.float32)

    # binary expanded to full object grid; diagonal dummy 0.5
    bf = np.full((B, O, O, P2), 0.5, dtype=np.float32)
    io, jo = np.meshgrid(np.arange(O), np.arange(O), indexing="ij")
    mask = io != jo
    bf[:, io[mask], jo[mask], :] = binary[:, io[mask],
                                          (jo - (jo > io))[mask], :]

    # x tables with partition = k_local (replicated 4x), free = indices
    xb_t = np.ascontiguousarray(
        bf.reshape(GRID, P2).T)                            # [32, 4096]
    xb_in = np.tile(xb_t, (4, 1))                          # [128, 4096]
    bfT = np.ascontiguousarray(np.swapaxes(bf, 1, 2))      # [B,o2,o1,P2]
    xbT_in = np.tile(bfT.reshape(GRID, P2).T, (4, 1))      # [128, 4096]
    xu_t = np.ascontiguousarray(
        unary.reshape(B * O, P1).T)                        # [32, 256]
    xu_in = np.tile(xu_t, (4, 1))
    xn_t = np.ascontiguousarray(nullary.T)                 # [32, 16]
    xn_in = np.tile(xn_t, (4, 1))

    bdu = np.zeros((128, 36), dtype=np.float32)
    for g in range(3):
        for p in range(128):
            bdu[p, 12 * g + 4 * g + p // 32] = 1.0

    kstart = [96, 128, 32, 64, 0]   # pass t -> k block start
    p_idx = np.arange(128)
    in_maps = []
    for core in range(N_CORES):
        cs = core * CPC
        scl = np.empty((128, 15), dtype=np.float32)
        bia = np.empty((128, 15), dtype=np.float32)
        for g in range(3):
            cc = cs + p_idx // 32          # conjunct per partition
            kk = p_idx % 32
            for tpass in range(5):
                scl[:, g * 5 + tpass] = d[g, cc, kstart[tpass] + kk]
                bia[:, g * 5 + tpass] = e[g, cc, kstart[tpass] + kk]
        lnokv = np.empty((12, 1), dtype=np.float32)
        for p in range(12):
            lnokv[p, 0] = np.float32(np.log(ok[p // 4, cs + p % 4]))
        in_maps.append({
            "xb": xb_in, "xbT": xbT_in, "xu": xu_in, "xn": xn_in,
            "scl": scl, "bia": bia, "lnok": lnokv,
            "bdu": bdu,
        })
    return ak.astype(np.float32), ok.astype(np.float32), in_maps


def _host_post(u_grid, ak, ok):
    """u_grid [R, C, GRID] f32: per-conjunct 1 - ok*conj from the cores."""
    disj = (1.0 - np.prod(u_grid, axis=1)).reshape(R, B, O, O)
    a_ = np.repeat(np.arange(O), O - 1)
    bbi = np.tile(np.arange(O - 1), O)
    bb = bbi + (bbi >= a_)
    rules = disj[:, :, a_, bb]                             # [R,B,NPERM]
    rules = np.moveaxis(rules, 0, -1).reshape(B, O, O - 1, R)
    nullary_rules = (1.0 - np.prod(1.0 - rules[..., 0], axis=(1, 2))
                     )[:, None].astype(np.float32)
    unary_rules = (1.0 - np.prod(1.0 - rules[..., 1], axis=2)
                   )[:, :, None].astype(np.float32)
    binary_rules = rules[..., 2:3].astype(np.float32)
    return nullary_rules, unary_rules, binary_rules, ak, ok


def kernel(nullary, unary, binary, and_kernel, or_kernel, temperature):
    nullary = np.asarray(nullary, dtype=np.float32)
    unary = np.asarray(unary, dtype=np.float32)
    binary = np.asarray(binary, dtype=np.float32)
    and_kernel = np.asarray(and_kernel, dtype=np.float32)
    or_kernel = np.asarray(or_kernel, dtype=np.float32)
    temperature = np.asarray(temperature, dtype=np.float32)

    ak, ok, in_maps = _host_prep(nullary, unary, binary,
                                 and_kernel, or_kernel, temperature)

    import os
    if "nc" not in _cache:
        _cache["nc"] = _build_bass()
    from concourse.bass_utils import run_bass_kernel_spmd
    res = run_bass_kernel_spmd(_cache["nc"], in_maps,
                               core_ids=list(range(N_CORES)),
                               trace=bool(os.environ.get("KERNEL_TRACE")))
    _cache["last_results"] = res
    u_grid = np.empty((R, C, GRID), dtype=np.float32)
    for i, r in enumerate(res.results):
        negt = r["uo"]                    # -t; rc = r*4 + cc_local
        for rr in range(R):
            u_grid[rr, i * CPC:(i + 1) * CPC] = (
                1.0 + negt[4 * rr:4 * rr + 4])
    return _host_post(u_grid, ak, ok)


# revision 33
# speedup vs baseline: 1.0018x; 1.0018x over previous
"""Trainium2 Bass kernel for nn_DNF_21071109554827 (soft DNF rule network).

Math: the conjunct product prod_k(x_k*a0 + (1-x_k)*a1 + a2) over NUM_IN=160
inputs is separable in log space into 5 blocks (nullary / unary slot a /
unary slot b / binary (a,b) / binary (b,a)).  The per-block tables are
computed over the full 16x16 object grid so every gather is an affine
access pattern; diagonal (invalid) entries are dropped on the host.

Sharding: the 96 (rule r, conjunct c) pairs are split across 8 cores
(3 rules x 4 conjuncts each).  Each core computes partial
sum_{c in core} ln(1 - ok*conj) -> [3, B*16*16]; the host adds partials
across cores (= product over all 32 conjuncts), finishes the
probabilistic sums, and returns full outputs.

Device layout per core:
  partition = (rc_local//...) packed as 4 rc x 32 k = 128 lanes
  ACT computes Ln(x*d + e) with per-partition scale/bias (d, e) --
  one fused instruction per (table, rc-group, column-split).
  PE reduces over k (block-diag ones lhsT) and accumulates all 5 tables
  straight into U[12, 512] PSUM chunks, using broadcast access patterns
  to expand the small tables over the (a, bb) grid.
  ACT: conj = Exp(U); DVE: u = 1 - ok*conj (exact fp32) -> DMA out.
  Host multiplies u across all 32 conjuncts (fp32, matching the
  reference's product semantics bit-for-bit in the common tiny-t case).
"""

import numpy as np

# ---------------- static config (hardcoded; must match the problem) -------
B, O, V = 16, 16, 2
P0 = P1 = P2 = 32
R, C = 3, 32
NPERM = O * (O - 1)
NUM_IN = P0 + V * P1 + V * (V - 1) * P2  # 160
N_CORES = 8
CPC = C // N_CORES        # conjuncts per core = 4
GRID = B * O * O          # 4096 free columns (b, a, bb)
NSPLIT = 2                # column splits for ACT/DMA pipelining
HALF = GRID // NSPLIT
NCHUNK = GRID // 512      # 8 PSUM chunks of 512 cols

_cache = {}


def _split_waits(nc, cap=1):
    """The container's walrus rejects instructions with more than ~1 sync
    wait.  Move excess waits onto injected same-engine NOPs placed right
    before the instruction (same serial engine stream -> semantics kept)."""
    import concourse.mybir as mb

    uid = [0]
    for f in nc.m.functions:
        for blk in f.blocks:
            out = []
            changed = False
            for inst in blk.instructions:
                si = inst.sync_info
                waits = list(si.on_wait) if si is not None else []
                if len(waits) > cap:
                    changed = True
                    for w in waits[:-cap]:
                        nop = mb.InstNoOp(
                            name=f"waitsplit-{uid[0]}", ins=[], outs=[])
                        uid[0] += 1
                        nop.engine = inst.engine
                        nop.sync_info = mb.SyncInfo(on_wait=[w], on_update=[])
                        out.append(nop)
                    inst.sync_info = mb.SyncInfo(
                        on_wait=waits[-cap:], on_update=list(si.on_update))
                out.append(inst)
            if changed:
                blk.instructions = out


def _build_bass(debug=False, split=True):
    import concourse.bass as bass
    import concourse.mybir as mybir
    from concourse import tile

    fp32 = mybir.dt.float32
    AF = mybir.ActivationFunctionType

    nc = bass.Bass()
    if debug:
        dbg_conj = nc.dram_tensor("dbg_conj", [12, GRID], fp32,
                                  kind="ExternalOutput")
        dbg_l3 = nc.dram_tensor("dbg_l3", [128, GRID], fp32,
                                kind="ExternalOutput")
    bf16 = mybir.dt.bfloat16
    xb = nc.dram_tensor("xb", [128, GRID], bf16, kind="ExternalInput")
    xbT = nc.dram_tensor("xbT", [128, GRID], bf16, kind="ExternalInput")
    xu = nc.dram_tensor("xu", [128, B * O], fp32, kind="ExternalInput")
    xn = nc.dram_tensor("xn", [128, B], fp32, kind="ExternalInput")
    scl = nc.dram_tensor("scl", [128, 15], fp32, kind="ExternalInput")
    bia = nc.dram_tensor("bia", [128, 15], fp32, kind="ExternalInput")
    lnok = nc.dram_tensor("lnok", [12, 1], fp32, kind="ExternalInput")
    bdu = nc.dram_tensor("bdu", [128, 36], fp32, kind="ExternalInput")
    uo = nc.dram_tensor("uo", [12, GRID], fp32, kind="ExternalOutput")

    with tile.TileContext(nc) as tc:
        with (
            tc.tile_pool(name="const", bufs=1) as cpool,
            tc.tile_pool(name="big", bufs=1) as bpool,
            tc.tile_pool(name="psumU", bufs=6, space="PSUM") as ppool,
            tc.tile_pool(name="psumC", bufs=1, space="PSUM") as qpool,
            tc.tile_pool(name="tail", bufs=4) as tpool,
        ):
            # ---- load constants / inputs ----
            s_xu = cpool.tile([128, B * O], fp32, tag="xu")
            s_xn = cpool.tile([128, B], fp32, tag="xn")
            s_scl = cpool.tile([128, 15], fp32, tag="scl")
            s_bia = cpool.tile([128, 15], fp32, tag="bia")
            s_lnok = cpool.tile([12, 1], fp32, tag="lnok")
            s_bdu = cpool.tile([128, 36], fp32, tag="bdu")
            # warm the ACT Ln/Exp PWP tables before anything else
            warm = cpool.tile([1, 1], fp32, tag="warm", name="warm")
            nc.vector.memset(warm[:], 1.0)
            nc.scalar.activation(warm[:], warm[:], AF.Ln)
            nc.scalar.activation(warm[:], warm[:], AF.Exp)
            nc.sync.dma_start(s_scl[:], scl[:])
            nc.sync.dma_start(s_bia[:], bia[:])
            nc.sync.dma_start(s_xu[:], xu[:])
            nc.sync.dma_start(s_xn[:], xn[:])
            nc.sync.dma_start(s_lnok[:], lnok[:])
            nc.sync.dma_start(s_bdu[:], bdu[:])

            # ---- big Ln tables (flat grids; xbT pre-transposed on host) ----
            # pass t: 0=S3'(k96), 1=S4'(k128), 2=S1(k32), 3=S2(k64), 4=S0(k0)
            s_xb = bpool.tile([128, GRID], bf16, tag="xb")
            s_xbT = bpool.tile([128, GRID], bf16, tag="xbT")
            L3 = [[None] * 3 for _ in range(NSPLIT)]
            L4 = [[None] * 3 for _ in range(NSPLIT)]
            compact_done = False
            for h in range(NSPLIT):
                sl = slice(h * HALF, (h + 1) * HALF)
                nc.sync.dma_start(s_xb[:, sl], xb[:, sl])
                nc.gpsimd.dma_start(s_xbT[:, sl], xbT[:, sl])
                for g in range(3):
                    t3 = bpool.tile([128, HALF], fp32, tag=f"L3_{g}_{h}")
                    t4 = bpool.tile([128, HALF], fp32, tag=f"L4_{g}_{h}")
                    j0 = g * 5 + 0
                    j1 = g * 5 + 1
                    nc.scalar.activation(
                        t3[:], s_xb[:, sl], AF.Ln,
                        bias=s_bia[:, j0:j0 + 1], scale=s_scl[:, j0:j0 + 1])
                    nc.scalar.activation(
                        t4[:], s_xbT[:, sl], AF.Ln,
                        bias=s_bia[:, j1:j1 + 1], scale=s_scl[:, j1:j1 + 1])
                    L3[h][g] = t3
                    L4[h][g] = t4
                if compact_done:
                    continue
                compact_done = True
                # ---- compact tables S1/S2/S0 -> E1 = exp(S0+S1+ln ok), E2
                u1 = qpool.tile([12, B * O], fp32, tag="U1")
                u2 = qpool.tile([12, B * O], fp32, tag="U2")
                for g in range(3):
                    t1 = cpool.tile([128, B * O], fp32, tag=f"L1_{g}")
                    t2 = cpool.tile([128, B * O], fp32, tag=f"L2_{g}")
                    t0 = cpool.tile([128, B], fp32, tag=f"L0_{g}")
                    for t, (tt, xin) in enumerate(
                            [(t1, s_xu), (t2, s_xu), (t0, s_xn)], start=2):
                        j = g * 5 + t
                        nc.scalar.activation(
                            tt[:], xin[:], AF.Ln,
                            bias=s_bia[:, j:j + 1], scale=s_scl[:, j:j + 1])
                    w_g = s_bdu[:, 12 * g:12 * g + 12]
                    nc.tensor.matmul(u1[:], w_g, t1[:],
                                     start=(g == 0), stop=False)
                    r0 = t0[:, :, None].broadcast_to((128, B, O))
                    nc.tensor.matmul(
                        u1[:].rearrange("p (b o) -> p b o", b=B), w_g, r0,
                        start=False, stop=(g == 2))
                    nc.tensor.matmul(u2[:], w_g, t2[:],
                                     start=(g == 0), stop=(g == 2))
                e1 = cpool.tile([12, B * O], fp32, tag="E1")
                e2 = cpool.tile([12, B * O], fp32, tag="E2")
                nc.scalar.activation(e1[:], u1[:], AF.Exp, bias=s_lnok[:])
                nc.scalar.activation(e2[:], u2[:], AF.Exp)

            # ---- U accumulation: g-outer so PE keeps each weight loaded ----
            U = [None] * NCHUNK
            for h in range(NSPLIT):
                for g in range(3):
                    w_g = s_bdu[:, 12 * g:12 * g + 12]
                    for m in range(h * NCHUNK // NSPLIT,
                                   (h + 1) * NCHUNK // NSPLIT):
                        off = m * 512 - h * HALF
                        if g == 0:
                            U[m] = ppool.tile([12, 512], fp32, tag="U", name=f"U{m}")
                        nc.tensor.matmul(
                            U[m][:], w_g, L3[h][g][:, off:off + 512],
                            start=(g == 0), stop=False)
                        nc.tensor.matmul(
                            U[m][:], w_g, L4[h][g][:, off:off + 512],
                            start=False, stop=(g == 2))
                # ---- tails for this half ----
                for m in range(h * NCHUNK // NSPLIT,
                               (h + 1) * NCHUNK // NSPLIT):
                    bpair = slice(2 * m, 2 * m + 2)
                    nm = tpool.tile([12, 512], fp32, tag="nm")
                    r1 = e1[:].rearrange("p (b o) -> p b o", b=B)
                    r1 = r1[:, bpair, :, None].broadcast_to((12, 2, O, O))
                    r2 = e2[:].rearrange("p (b o) -> p b o", b=B)
                    r2 = r2[:, bpair, None, :].broadcast_to((12, 2, O, O))
                    nc.vector.tensor_tensor(
                        nm[:].rearrange("p (b x y) -> p b x y", b=2, x=O),
                        r1, r2, mybir.AluOpType.mult)
                    e34 = tpool.tile([12, 512], fp32, tag="e34")
                    nc.scalar.activation(e34[:], U[m][:], AF.Exp)
                    w = tpool.tile([12, 512], fp32, tag="w")
                    nc.vector.scalar_tensor_tensor(
                        w[:], e34[:], -1.0, nm[:],
                        mybir.AluOpType.mult, mybir.AluOpType.mult)
                    nc.sync.dma_start(uo[:, m * 512:(m + 1) * 512], w[:])

    if split:
        _split_waits(nc)
    return nc


def _host_prep(nullary, unary, binary, and_kernel, or_kernel, temperature):
    """Everything cheap: softmax/sigmoid, tables, per-core input maps."""
    t = np.float64(temperature.reshape(-1)[0])
    akd = and_kernel.astype(np.float64) / t
    akd = akd - akd.max(axis=-1, keepdims=True)
    eak = np.exp(akd)
    ak = (eak / eak.sum(axis=-1, keepdims=True))          # [R,C,160,3] f64
    ok = 1.0 / (1.0 + np.exp(-or_kernel.astype(np.float64) / t))  # [R,C]

    d = (ak[..., 0] - ak[..., 1]).astype(np.float32)      # [R,C,160]
    e = (ak[..., 1] + ak[..., 2]).astype(np.float32)

    # binary expanded to full object grid; diagonal dummy 0.5
    bf = np.full((B, O, O, P2), 0.5, dtype=np.float32)
    io, jo = np.meshgrid(np.arange(O), np.arange(O), indexing="ij")
    mask = io != jo
    bf[:, io[mask], jo[mask], :] = binary[:, io[mask],
                                          (jo - (jo > io))[mask], :]

    # x tables with partition = k_local (replicated 4x), free = indices
    import ml_dtypes
    xb_t = np.ascontiguousarray(
        bf.reshape(GRID, P2).T)                            # [32, 4096]
    xb_in = np.tile(xb_t, (4, 1)).astype(ml_dtypes.bfloat16)
    bfT = np.ascontiguousarray(np.swapaxes(bf, 1, 2))      # [B,o2,o1,P2]
    xbT_in = np.tile(bfT.reshape(GRID, P2).T,
                     (4, 1)).astype(ml_dtypes.bfloat16)
    xu_t = np.ascontiguousarray(
        unary.reshape(B * O, P1).T)                        # [32, 256]
    xu_in = np.tile(xu_t, (4, 1))
    xn_t = np.ascontiguousarray(nullary.T)                 # [32, 16]
    xn_in = np.tile(xn_t, (4, 1))

    bdu = np.zeros((128, 36), dtype=np.float32)
    for g in range(3):
        for p in range(128):
            bdu[p, 12 * g + 4 * g + p // 32] = 1.0

    kstart = [96, 128, 32, 64, 0]   # pass t -> k block start
    p_idx = np.arange(128)
    in_maps = []
    for core in range(N_CORES):
        cs = core * CPC
        scl = np.empty((128, 15), dtype=np.float32)
        bia = np.empty((128, 15), dtype=np.float32)
        for g in range(3):
            cc = cs + p_idx // 32          # conjunct per partition
            kk = p_idx % 32
            for tpass in range(5):
                scl[:, g * 5 + tpass] = d[g, cc, kstart[tpass] + kk]
                bia[:, g * 5 + tpass] = e[g, cc, kstart[tpass] + kk]
        lnokv = np.empty((12, 1), dtype=np.float32)
        for p in range(12):
            lnokv[p, 0] = np.float32(np.log(ok[p // 4, cs + p % 4]))
        in_maps.append({
            "xb": xb_in, "xbT": xbT_in, "xu": xu_in, "xn": xn_in,
            "scl": scl, "bia": bia, "lnok": lnokv,
            "bdu": bdu,
        })
    return ak.astype(np.float32), ok.astype(np.float32), in_maps


def _host_post(u_grid, ak, ok):
    """u_grid [R, C, GRID] f32: per-conjunct 1 - ok*conj from the cores."""
    disj = (1.0 - np.prod(u_grid, axis=1)).reshape(R, B, O, O)
    a_ = np.repeat(np.arange(O), O - 1)
    bbi = np.tile(np.arange(O - 1), O)
    bb = bbi + (bbi >= a_)
    rules = disj[:, :, a_, bb]                             # [R,B,NPERM]
    rules = np.moveaxis(rules, 0, -1).reshape(B, O, O - 1, R)
    nullary_rules = (1.0 - np.prod(1.0 - rules[..., 0], axis=(1, 2))
                     )[:, None].astype(np.float32)
    unary_rules = (1.0 - np.prod(1.0 - rules[..., 1], axis=2)
                   )[:, :, None].astype(np.float32)
    binary_rules = rules[..., 2:3].astype(np.float32)
    return nullary_rules, unary_rules, binary_rules, ak, ok


def kernel(nullary, unary, binary, and_kernel, or_kernel, temperature):
    nullary = np.asarray(nullary, dtype=np.float32)
    unary = np.asarray(unary, dtype=np.float32)
    binary = np.asarray(binary, dtype=np.float32)
    and_kernel = np.asarray(and_kernel, dtype=np.float32)
    or_kernel = np.asarray(or_kernel, dtype=np.float32)
    temperature = np.asarray(temperature, dtype=np.float32)

    ak, ok, in_maps = _host_prep(nullary, unary, binary,
                                 and_kernel, or_kernel, temperature)

    import os
    if "nc" not in _cache:
        _cache["nc"] = _build_bass()
    from concourse.bass_utils import run_bass_kernel_spmd
    res = run_bass_kernel_spmd(_cache["nc"], in_maps,
                               core_ids=list(range(N_CORES)),
                               trace=bool(os.environ.get("KERNEL_TRACE")))
    _cache["last_results"] = res
    u_grid = np.empty((R, C, GRID), dtype=np.float32)
    for i, r in enumerate(res.results):
        negt = r["uo"]                    # -t; rc = r*4 + cc_local
        for rr in range(R):
            u_grid[rr, i * CPC:(i + 1) * CPC] = (
                1.0 + negt[4 * rr:4 * rr + 4])
    return _host_post(u_grid, ak, ok)


# revision 34
# speedup vs baseline: 1.0329x; 1.0310x over previous
"""Trainium2 Bass kernel for nn_DNF_21071109554827 (soft DNF rule network).

Math: the conjunct product prod_k(x_k*a0 + (1-x_k)*a1 + a2) over NUM_IN=160
inputs is separable in log space into 5 blocks (nullary / unary slot a /
unary slot b / binary (a,b) / binary (b,a)).  The per-block tables are
computed over the full 16x16 object grid so every gather is an affine
access pattern; diagonal (invalid) entries are dropped on the host.

Sharding: the 96 (rule r, conjunct c) pairs are split across 8 cores
(3 rules x 4 conjuncts each).  Each core computes partial
sum_{c in core} ln(1 - ok*conj) -> [3, B*16*16]; the host adds partials
across cores (= product over all 32 conjuncts), finishes the
probabilistic sums, and returns full outputs.

Device layout per core:
  partition = (rc_local//...) packed as 4 rc x 32 k = 128 lanes
  ACT computes Ln(x*d + e) with per-partition scale/bias (d, e) --
  one fused instruction per (table, rc-group, column-split).
  PE reduces over k (block-diag ones lhsT) and accumulates all 5 tables
  straight into U[12, 512] PSUM chunks, using broadcast access patterns
  to expand the small tables over the (a, bb) grid.
  ACT: conj = Exp(U); DVE: u = 1 - ok*conj (exact fp32) -> DMA out.
  Host multiplies u across all 32 conjuncts (fp32, matching the
  reference's product semantics bit-for-bit in the common tiny-t case).
"""

import numpy as np

# ---------------- static config (hardcoded; must match the problem) -------
B, O, V = 16, 16, 2
P0 = P1 = P2 = 32
R, C = 3, 32
NPERM = O * (O - 1)
NUM_IN = P0 + V * P1 + V * (V - 1) * P2  # 160
N_CORES = 8
CPC = C // N_CORES        # conjuncts per core = 4
GRID = B * O * O          # 4096 free columns (b, a, bb)
NSPLIT = 2                # column splits for ACT/DMA pipelining
HALF = GRID // NSPLIT
NCHUNK = GRID // 512      # 8 PSUM chunks of 512 cols

_cache = {}


def _split_waits(nc, cap=1):
    """The container's walrus rejects instructions with more than ~1 sync
    wait.  Move excess waits onto injected same-engine NOPs placed right
    before the instruction (same serial engine stream -> semantics kept)."""
    import concourse.mybir as mb

    uid = [0]
    for f in nc.m.functions:
        for blk in f.blocks:
            out = []
            changed = False
            for inst in blk.instructions:
                si = inst.sync_info
                waits = list(si.on_wait) if si is not None else []
                if len(waits) > cap:
                    changed = True
                    for w in waits[:-cap]:
                        nop = mb.InstNoOp(
                            name=f"waitsplit-{uid[0]}", ins=[], outs=[])
                        uid[0] += 1
                        nop.engine = inst.engine
                        nop.sync_info = mb.SyncInfo(on_wait=[w], on_update=[])
                        out.append(nop)
                    inst.sync_info = mb.SyncInfo(
                        on_wait=waits[-cap:], on_update=list(si.on_update))
                out.append(inst)
            if changed:
                blk.instructions = out


def _build_bass(debug=False, split=True):
    import concourse.bass as bass
    import concourse.mybir as mybir
    from concourse import tile

    fp32 = mybir.dt.float32
    AF = mybir.ActivationFunctionType

    nc = bass.Bass()
    if debug:
        dbg_conj = nc.dram_tensor("dbg_conj", [12, GRID], fp32,
                                  kind="ExternalOutput")
        dbg_l3 = nc.dram_tensor("dbg_l3", [128, GRID], fp32,
                                kind="ExternalOutput")
    bf16 = mybir.dt.bfloat16
    xb = nc.dram_tensor("xb", [128, GRID], bf16, kind="ExternalInput")
    xbT = nc.dram_tensor("xbT", [128, GRID], bf16, kind="ExternalInput")
    xu = nc.dram_tensor("xu", [128, B * O], fp32, kind="ExternalInput")
    xn = nc.dram_tensor("xn", [128, B], fp32, kind="ExternalInput")
    scl = nc.dram_tensor("scl", [128, 15], fp32, kind="ExternalInput")
    bia = nc.dram_tensor("bia", [128, 15], fp32, kind="ExternalInput")
    lnok = nc.dram_tensor("lnok", [12, 1], fp32, kind="ExternalInput")
    bdu = nc.dram_tensor("bdu", [128, 36], fp32, kind="ExternalInput")
    uo = nc.dram_tensor("uo", [12, GRID], fp32, kind="ExternalOutput")

    with tile.TileContext(nc) as tc:
        with (
            tc.tile_pool(name="const", bufs=1) as cpool,
            tc.tile_pool(name="big", bufs=1) as bpool,
            tc.tile_pool(name="psumU", bufs=6, space="PSUM") as ppool,
            tc.tile_pool(name="psumC", bufs=1, space="PSUM") as qpool,
            tc.tile_pool(name="tail", bufs=4) as tpool,
        ):
            # ---- load constants / inputs ----
            s_xu = cpool.tile([128, B * O], fp32, tag="xu")
            s_xn = cpool.tile([128, B], fp32, tag="xn")
            s_scl = cpool.tile([128, 15], fp32, tag="scl")
            s_bia = cpool.tile([128, 15], fp32, tag="bia")
            s_lnok = cpool.tile([12, 1], fp32, tag="lnok")
            s_bdu = cpool.tile([128, 36], fp32, tag="bdu")
            # warm the ACT Ln/Exp PWP tables before anything else
            warm = cpool.tile([1, 1], fp32, tag="warm", name="warm")
            nc.vector.memset(warm[:], 1.0)
            nc.scalar.activation(warm[:], warm[:], AF.Ln)
            nc.scalar.activation(warm[:], warm[:], AF.Exp)
            # small constants on the gpsimd queue; grids split across both
            nc.gpsimd.dma_start(s_scl[:], scl[:])
            nc.gpsimd.dma_start(s_bia[:], bia[:])
            nc.gpsimd.dma_start(s_xu[:], xu[:])
            nc.gpsimd.dma_start(s_xn[:], xn[:])
            nc.gpsimd.dma_start(s_lnok[:], lnok[:])
            nc.gpsimd.dma_start(s_bdu[:], bdu[:])

            s_xb = bpool.tile([128, GRID], bf16, tag="xb")
            s_xbT = bpool.tile([128, GRID], bf16, tag="xbT")
            L3 = [[None] * 3 for _ in range(NSPLIT)]
            L4 = [[None] * 3 for _ in range(NSPLIT)]
            QS = HALF // 2
            for h in range(NSPLIT):
                q0 = slice(h * HALF, h * HALF + QS)
                q1 = slice(h * HALF + QS, (h + 1) * HALF)
                nc.sync.dma_start(s_xb[:, q0], xb[:, q0])
                nc.gpsimd.dma_start(s_xb[:, q1], xb[:, q1])
                nc.sync.dma_start(s_xbT[:, q0], xbT[:, q0])
                nc.gpsimd.dma_start(s_xbT[:, q1], xbT[:, q1])
                sl = slice(h * HALF, (h + 1) * HALF)
                for g in range(3):
                    t3 = bpool.tile([128, HALF], fp32, tag=f"L3_{g}_{h}")
                    t4 = bpool.tile([128, HALF], fp32, tag=f"L4_{g}_{h}")
                    j0 = g * 5 + 0
                    j1 = g * 5 + 1
                    nc.scalar.activation(
                        t3[:], s_xb[:, sl], AF.Ln,
                        bias=s_bia[:, j0:j0 + 1], scale=s_scl[:, j0:j0 + 1])
                    nc.scalar.activation(
                        t4[:], s_xbT[:, sl], AF.Ln,
                        bias=s_bia[:, j1:j1 + 1], scale=s_scl[:, j1:j1 + 1])
                    L3[h][g] = t3
                    L4[h][g] = t4
                if h == 0:
                    # compact tables S1/S2/S0 -> E1 = exp(S0+S1+ln ok), E2
                    u1 = qpool.tile([12, B * O], fp32, tag="U1")
                    u2 = qpool.tile([12, B * O], fp32, tag="U2")
                    for g in range(3):
                        t1 = cpool.tile([128, B * O], fp32, tag=f"L1_{g}")
                        t2 = cpool.tile([128, B * O], fp32, tag=f"L2_{g}")
                        t0 = cpool.tile([128, B], fp32, tag=f"L0_{g}")
                        for t, (tt, xin) in enumerate(
                                [(t1, s_xu), (t2, s_xu), (t0, s_xn)], start=2):
                            j = g * 5 + t
                            nc.scalar.activation(
                                tt[:], xin[:], AF.Ln,
                                bias=s_bia[:, j:j + 1], scale=s_scl[:, j:j + 1])
                        w_g = s_bdu[:, 12 * g:12 * g + 12]
                        nc.tensor.matmul(u1[:], w_g, t1[:],
                                         start=(g == 0), stop=False)
                        r0 = t0[:, :, None].broadcast_to((128, B, O))
                        nc.tensor.matmul(
                            u1[:].rearrange("p (b o) -> p b o", b=B), w_g, r0,
                            start=False, stop=(g == 2))
                        nc.tensor.matmul(u2[:], w_g, t2[:],
                                         start=(g == 0), stop=(g == 2))
                    e1 = cpool.tile([12, B * O], fp32, tag="E1")
                    e2 = cpool.tile([12, B * O], fp32, tag="E2")
                    nc.scalar.activation(e1[:], u1[:], AF.Exp, bias=s_lnok[:])
                    nc.scalar.activation(e2[:], u2[:], AF.Exp)

                # chunk-major U accumulation + tail, per half
                for m in range(h * NCHUNK // NSPLIT,
                               (h + 1) * NCHUNK // NSPLIT):
                    off = m * 512 - h * HALF
                    u = ppool.tile([12, 512], fp32, tag="U", name=f"U{m}")
                    for g in range(3):
                        w_g = s_bdu[:, 12 * g:12 * g + 12]
                        nc.tensor.matmul(
                            u[:], w_g, L3[h][g][:, off:off + 512],
                            start=(g == 0), stop=False)
                        nc.tensor.matmul(
                            u[:], w_g, L4[h][g][:, off:off + 512],
                            start=False, stop=(g == 2))
                    bpair = slice(2 * m, 2 * m + 2)
                    nm = tpool.tile([12, 512], fp32, tag="nm")
                    r1 = e1[:].rearrange("p (b o) -> p b o", b=B)
                    r1 = r1[:, bpair, :, None].broadcast_to((12, 2, O, O))
                    r2 = e2[:].rearrange("p (b o) -> p b o", b=B)
                    r2 = r2[:, bpair, None, :].broadcast_to((12, 2, O, O))
                    nc.vector.tensor_tensor(
                        nm[:].rearrange("p (b x y) -> p b x y", b=2, x=O),
                        r1, r2, mybir.AluOpType.mult)
                    e34 = tpool.tile([12, 512], fp32, tag="e34")
                    nc.scalar.activation(e34[:], u[:], AF.Exp)
                    w = tpool.tile([12, 512], fp32, tag="w")
                    nc.vector.scalar_tensor_tensor(
                        w[:], e34[:], -1.0, nm[:],
                        mybir.AluOpType.mult, mybir.AluOpType.mult)
                    nc.sync.dma_start(uo[:, m * 512:(m + 1) * 512], w[:])

    if split:
        _split_waits(nc)
    return nc


def _host_prep(nullary, unary, binary, and_kernel, or_kernel, temperature):
    """Everything cheap: softmax/sigmoid, tables, per-core input maps."""
    t = np.float64(temperature.reshape(-1)[0])
    akd = and_kernel.astype(np.float64) / t
    akd = akd - akd.max(axis=-1, keepdims=True)
    eak = np.exp(akd)
    ak = (eak / eak.sum(axis=-1, keepdims=True))          # [R,C,160,3] f64
    ok = 1.0 / (1.0 + np.exp(-or_kernel.astype(np.float64) / t))  # [R,C]

    d = (ak[..., 0] - ak[..., 1]).astype(np.float32)      # [R,C,160]
    e = (ak[..., 1] + ak[..., 2]).astype(np.float32)

    # binary expanded to full object grid; diagonal dummy 0.5
    bf = np.full((B, O, O, P2), 0.5, dtype=np.float32)
    io, jo = np.meshgrid(np.arange(O), np.arange(O), indexing="ij")
    mask = io != jo
    bf[:, io[mask], jo[mask], :] = binary[:, io[mask],
                                          (jo - (jo > io))[mask], :]

    # x tables with partition = k_local (replicated 4x), free = indices
    import ml_dtypes
    xb_t = np.ascontiguousarray(
        bf.reshape(GRID, P2).T)                            # [32, 4096]
    xb_in = np.tile(xb_t, (4, 1)).astype(ml_dtypes.bfloat16)
    bfT = np.ascontiguousarray(np.swapaxes(bf, 1, 2))      # [B,o2,o1,P2]
    xbT_in = np.tile(bfT.reshape(GRID, P2).T,
                     (4, 1)).astype(ml_dtypes.bfloat16)
    xu_t = np.ascontiguousarray(
        unary.reshape(B * O, P1).T)                        # [32, 256]
    xu_in = np.tile(xu_t, (4, 1))
    xn_t = np.ascontiguousarray(nullary.T)                 # [32, 16]
    xn_in = np.tile(xn_t, (4, 1))

    bdu = np.zeros((128, 36), dtype=np.float32)
    for g in range(3):
        for p in range(128):
            bdu[p, 12 * g + 4 * g + p // 32] = 1.0

    kstart = [96, 128, 32, 64, 0]   # pass t -> k block start
    p_idx = np.arange(128)
    in_maps = []
    for core in range(N_CORES):
        cs = core * CPC
        scl = np.empty((128, 15), dtype=np.float32)
        bia = np.empty((128, 15), dtype=np.float32)
        for g in range(3):
            cc = cs + p_idx // 32          # conjunct per partition
            kk = p_idx % 32
            for tpass in range(5):
                scl[:, g * 5 + tpass] = d[g, cc, kstart[tpass] + kk]
                bia[:, g * 5 + tpass] = e[g, cc, kstart[tpass] + kk]
        lnokv = np.empty((12, 1), dtype=np.float32)
        for p in range(12):
            lnokv[p, 0] = np.float32(np.log(ok[p // 4, cs + p % 4]))
        in_maps.append({
            "xb": xb_in, "xbT": xbT_in, "xu": xu_in, "xn": xn_in,
            "scl": scl, "bia": bia, "lnok": lnokv,
            "bdu": bdu,
        })
    return ak.astype(np.float32), ok.astype(np.float32), in_maps


def _host_post(u_grid, ak, ok):
    """u_grid [R, C, GRID] f32: per-conjunct 1 - ok*conj from the cores."""
    disj = (1.0 - np.prod(u_grid, axis=1)).reshape(R, B, O, O)
    a_ = np.repeat(np.arange(O), O - 1)
    bbi = np.tile(np.arange(O - 1), O)
    bb = bbi + (bbi >= a_)
    rules = disj[:, :, a_, bb]                             # [R,B,NPERM]
    rules = np.moveaxis(rules, 0, -1).reshape(B, O, O - 1, R)
    nullary_rules = (1.0 - np.prod(1.0 - rules[..., 0], axis=(1, 2))
                     )[:, None].astype(np.float32)
    unary_rules = (1.0 - np.prod(1.0 - rules[..., 1], axis=2)
                   )[:, :, None].astype(np.float32)
    binary_rules = rules[..., 2:3].astype(np.float32)
    return nullary_rules, unary_rules, binary_rules, ak, ok


def kernel(nullary, unary, binary, and_kernel, or_kernel, temperature):
    nullary = np.asarray(nullary, dtype=np.float32)
    unary = np.asarray(unary, dtype=np.float32)
    binary = np.asarray(binary, dtype=np.float32)
    and_kernel = np.asarray(and_kernel, dtype=np.float32)
    or_kernel = np.asarray(or_kernel, dtype=np.float32)
    temperature = np.asarray(temperature, dtype=np.float32)

    ak, ok, in_maps = _host_prep(nullary, unary, binary,
                                 and_kernel, or_kernel, temperature)

    import os
    if "nc" not in _cache:
        _cache["nc"] = _build_bass()
    from concourse.bass_utils import run_bass_kernel_spmd
    res = run_bass_kernel_spmd(_cache["nc"], in_maps,
                               core_ids=list(range(N_CORES)),
                               trace=bool(os.environ.get("KERNEL_TRACE")))
    _cache["last_results"] = res
    u_grid = np.empty((R, C, GRID), dtype=np.float32)
    for i, r in enumerate(res.results):
        negt = r["uo"]                    # -t; rc = r*4 + cc_local
        for rr in range(R):
            u_grid[rr, i * CPC:(i + 1) * CPC] = (
                1.0 + negt[4 * rr:4 * rr + 4])
    return _host_post(u_grid, ak, ok)


# revision 36
# speedup vs baseline: 1.3520x; 1.3089x over previous
"""Trainium2 Bass kernel for nn_DNF_21071109554827 (soft DNF rule network).

Math: the conjunct product prod_k(x_k*a0 + (1-x_k)*a1 + a2) over NUM_IN=160
inputs is separable in log space into 5 blocks (nullary / unary slot a /
unary slot b / binary (a,b) / binary (b,a)).  The per-block tables are
computed over the full 16x16 object grid so every gather is an affine
access pattern; diagonal (invalid) entries are dropped on the host.

Sharding: the 96 (rule r, conjunct c) pairs are split across 8 cores
(3 rules x 4 conjuncts each).  Each core computes partial
sum_{c in core} ln(1 - ok*conj) -> [3, B*16*16]; the host adds partials
across cores (= product over all 32 conjuncts), finishes the
probabilistic sums, and returns full outputs.

Device layout per core:
  partition = (rc_local//...) packed as 4 rc x 32 k = 128 lanes
  ACT computes Ln(x*d + e) with per-partition scale/bias (d, e) --
  one fused instruction per (table, rc-group, column-split).
  PE reduces over k (block-diag ones lhsT) and accumulates all 5 tables
  straight into U[12, 512] PSUM chunks, using broadcast access patterns
  to expand the small tables over the (a, bb) grid.
  ACT: conj = Exp(U); DVE: u = 1 - ok*conj (exact fp32) -> DMA out.
  Host multiplies u across all 32 conjuncts (fp32, matching the
  reference's product semantics bit-for-bit in the common tiny-t case).
"""

import numpy as np

# ---------------- static config (hardcoded; must match the problem) -------
B, O, V = 16, 16, 2
P0 = P1 = P2 = 32
R, C = 3, 32
NPERM = O * (O - 1)
NUM_IN = P0 + V * P1 + V * (V - 1) * P2  # 160
N_CORES = 8
CPC = C // N_CORES        # conjuncts per core = 4
GRID = B * O * O          # 4096 free columns (b, a, bb)
NSPLIT = 2                # column splits for ACT/DMA pipelining
HALF = GRID // NSPLIT
NCHUNK = GRID // 512      # 8 PSUM chunks of 512 cols

_cache = {}


def _split_waits(nc, cap=1):
    """The container's walrus rejects instructions with more than ~1 sync
    wait.  Move excess waits onto injected same-engine NOPs placed right
    before the instruction (same serial engine stream -> semantics kept)."""
    import concourse.mybir as mb

    uid = [0]
    for f in nc.m.functions:
        for blk in f.blocks:
            out = []
            changed = False
            for inst in blk.instructions:
                si = inst.sync_info
                waits = list(si.on_wait) if si is not None else []
                if len(waits) > cap:
                    changed = True
                    for w in waits[:-cap]:
                        nop = mb.InstNoOp(
                            name=f"waitsplit-{uid[0]}", ins=[], outs=[])
                        uid[0] += 1
                        nop.engine = inst.engine
                        nop.sync_info = mb.SyncInfo(on_wait=[w], on_update=[])
                        out.append(nop)
                    inst.sync_info = mb.SyncInfo(
                        on_wait=waits[-cap:], on_update=list(si.on_update))
                out.append(inst)
            if changed:
                blk.instructions = out


def _build_bass(debug=False, split=True):
    import concourse.bass as bass
    import concourse.mybir as mybir
    from concourse import tile

    fp32 = mybir.dt.float32
    AF = mybir.ActivationFunctionType

    nc = bass.Bass()
    if debug:
        dbg_conj = nc.dram_tensor("dbg_conj", [12, GRID], fp32,
                                  kind="ExternalOutput")
        dbg_l3 = nc.dram_tensor("dbg_l3", [128, GRID], fp32,
                                kind="ExternalOutput")
    bf16 = mybir.dt.bfloat16
    xb = nc.dram_tensor("xb", [128, GRID], bf16, kind="ExternalInput")
    xbT = nc.dram_tensor("xbT", [128, GRID], bf16, kind="ExternalInput")
    cst = nc.dram_tensor("cst", [128, 321], fp32, kind="ExternalInput")
    uo = nc.dram_tensor("uo", [12, GRID], fp32, kind="ExternalOutput")

    with tile.TileContext(nc) as tc:
        with (
            tc.tile_pool(name="const", bufs=1) as cpool,
            tc.tile_pool(name="big", bufs=1) as bpool,
            tc.tile_pool(name="psumU", bufs=6, space="PSUM") as ppool,
            tc.tile_pool(name="psumC", bufs=1, space="PSUM") as qpool,
            tc.tile_pool(name="tail", bufs=4) as tpool,
        ):
            # ---- one staged DMA for all small constants ----
            fp16 = mybir.dt.float16
            s_cst = cpool.tile([128, 321], fp32, tag="cst")
            # warm the ACT Ln/Exp PWP tables before anything else
            warm = cpool.tile([1, 1], fp32, tag="warm", name="warm")
            nc.vector.memset(warm[:], 1.0)
            nc.scalar.activation(warm[:], warm[:], AF.Ln)
            nc.scalar.activation(warm[:], warm[:], AF.Exp)
            nc.gpsimd.dma_start(s_cst[:], cst[:])
            s_scl = s_cst[:, 0:15]
            s_bia = s_cst[:, 15:30]
            s_xu = s_cst[:, 30:286]
            s_xn = s_cst[:, 286:302]
            s_lnok = s_cst[0:12, 302:303]
            s_bdu = s_cst[:, 303:321].bitcast(fp16)

            s_xb = bpool.tile([128, GRID], bf16, tag="xb")
            s_xbT = bpool.tile([128, GRID], bf16, tag="xbT")
            L3 = [[None] * 3 for _ in range(NSPLIT)]
            L4 = [[None] * 3 for _ in range(NSPLIT)]
            QS = HALF // 2
            for h in range(NSPLIT):
                q0 = slice(h * HALF, h * HALF + QS)
                q1 = slice(h * HALF + QS, (h + 1) * HALF)
                nc.sync.dma_start(s_xb[:, q0], xb[:, q0])
                nc.gpsimd.dma_start(s_xb[:, q1], xb[:, q1])
                nc.sync.dma_start(s_xbT[:, q0], xbT[:, q0])
                nc.gpsimd.dma_start(s_xbT[:, q1], xbT[:, q1])
                sl = slice(h * HALF, (h + 1) * HALF)
                for g in range(3):
                    t3 = bpool.tile([128, HALF], fp16, tag=f"L3_{g}_{h}")
                    t4 = bpool.tile([128, HALF], fp16, tag=f"L4_{g}_{h}")
                    j0 = g * 5 + 0
                    j1 = g * 5 + 1
                    nc.scalar.activation(
                        t3[:], s_xb[:, sl], AF.Ln,
                        bias=s_bia[:, j0:j0 + 1], scale=s_scl[:, j0:j0 + 1])
                    nc.scalar.activation(
                        t4[:], s_xbT[:, sl], AF.Ln,
                        bias=s_bia[:, j1:j1 + 1], scale=s_scl[:, j1:j1 + 1])
                    L3[h][g] = t3
                    L4[h][g] = t4
                if h == 0:
                    # compact tables S1/S2/S0 -> E1 = exp(S0+S1+ln ok), E2
                    u1 = qpool.tile([12, B * O], fp32, tag="U1")
                    u2 = qpool.tile([12, B * O], fp32, tag="U2")
                    for g in range(3):
                        t1 = cpool.tile([128, B * O], fp16, tag=f"L1_{g}")
                        t2 = cpool.tile([128, B * O], fp16, tag=f"L2_{g}")
                        t0 = cpool.tile([128, B], fp16, tag=f"L0_{g}")
                        for t, (tt, xin) in enumerate(
                                [(t1, s_xu), (t2, s_xu), (t0, s_xn)], start=2):
                            j = g * 5 + t
                            nc.scalar.activation(
                                tt[:], xin[:], AF.Ln,
                                bias=s_bia[:, j:j + 1], scale=s_scl[:, j:j + 1])
                        w_g = s_bdu[:, 12 * g:12 * g + 12]
                        nc.tensor.matmul(u1[:], w_g, t1[:],
                                         start=(g == 0), stop=False)
                        r0 = t0[:, :, None].broadcast_to((128, B, O))
                        nc.tensor.matmul(
                            u1[:].rearrange("p (b o) -> p b o", b=B), w_g, r0,
                            start=False, stop=(g == 2))
                        nc.tensor.matmul(u2[:], w_g, t2[:],
                                         start=(g == 0), stop=(g == 2))
                    e1 = cpool.tile([12, B * O], fp32, tag="E1")
                    e2 = cpool.tile([12, B * O], fp32, tag="E2")
                    nc.scalar.activation(e1[:], u1[:], AF.Exp, bias=s_lnok[:])
                    nc.scalar.activation(e2[:], u2[:], AF.Exp)

                # chunk-major U accumulation + tail, per half
                for m in range(h * NCHUNK // NSPLIT,
                               (h + 1) * NCHUNK // NSPLIT):
                    off = m * 512 - h * HALF
                    u = ppool.tile([12, 512], fp32, tag="U", name=f"U{m}")
                    for g in range(3):
                        w_g = s_bdu[:, 12 * g:12 * g + 12]
                        nc.tensor.matmul(
                            u[:], w_g, L3[h][g][:, off:off + 512],
                            start=(g == 0), stop=False)
                        nc.tensor.matmul(
                            u[:], w_g, L4[h][g][:, off:off + 512],
                            start=False, stop=(g == 2))
                    bpair = slice(2 * m, 2 * m + 2)
                    nm = tpool.tile([12, 512], fp32, tag="nm")
                    r1 = e1[:].rearrange("p (b o) -> p b o", b=B)
                    r1 = r1[:, bpair, :, None].broadcast_to((12, 2, O, O))
                    r2 = e2[:].rearrange("p (b o) -> p b o", b=B)
                    r2 = r2[:, bpair, None, :].broadcast_to((12, 2, O, O))
                    nc.vector.tensor_tensor(
                        nm[:].rearrange("p (b x y) -> p b x y", b=2, x=O),
                        r1, r2, mybir.AluOpType.mult)
                    e34 = tpool.tile([12, 512], fp32, tag="e34")
                    nc.scalar.activation(e34[:], u[:], AF.Exp)
                    w = tpool.tile([12, 512], fp32, tag="w")
                    nc.vector.scalar_tensor_tensor(
                        w[:], e34[:], -1.0, nm[:],
                        mybir.AluOpType.mult, mybir.AluOpType.mult)
                    nc.sync.dma_start(uo[:, m * 512:(m + 1) * 512], w[:])

    if split:
        _split_waits(nc)
    return nc


def _host_prep(nullary, unary, binary, and_kernel, or_kernel, temperature):
    """Everything cheap: softmax/sigmoid, tables, per-core input maps."""
    t = np.float64(temperature.reshape(-1)[0])
    akd = and_kernel.astype(np.float64) / t
    akd = akd - akd.max(axis=-1, keepdims=True)
    eak = np.exp(akd)
    ak = (eak / eak.sum(axis=-1, keepdims=True))          # [R,C,160,3] f64
    ok = 1.0 / (1.0 + np.exp(-or_kernel.astype(np.float64) / t))  # [R,C]

    d = (ak[..., 0] - ak[..., 1]).astype(np.float32)      # [R,C,160]
    e = (ak[..., 1] + ak[..., 2]).astype(np.float32)

    # binary expanded to full object grid; diagonal dummy 0.5
    bf = np.full((B, O, O, P2), 0.5, dtype=np.float32)
    io, jo = np.meshgrid(np.arange(O), np.arange(O), indexing="ij")
    mask = io != jo
    bf[:, io[mask], jo[mask], :] = binary[:, io[mask],
                                          (jo - (jo > io))[mask], :]

    # x tables with partition = k_local (replicated 4x), free = indices
    import ml_dtypes
    xb_t = np.ascontiguousarray(
        bf.reshape(GRID, P2).T)                            # [32, 4096]
    xb_in = np.tile(xb_t, (4, 1)).astype(ml_dtypes.bfloat16)
    bfT = np.ascontiguousarray(np.swapaxes(bf, 1, 2))      # [B,o2,o1,P2]
    xbT_in = np.tile(bfT.reshape(GRID, P2).T,
                     (4, 1)).astype(ml_dtypes.bfloat16)
    xu_t = np.ascontiguousarray(
        unary.reshape(B * O, P1).T)                        # [32, 256]
    xu_in = np.tile(xu_t, (4, 1))
    xn_t = np.ascontiguousarray(nullary.T)                 # [32, 16]
    xn_in = np.tile(xn_t, (4, 1))

    bdu = np.zeros((128, 36), dtype=np.float32)
    for g in range(3):
        for p in range(128):
            bdu[p, 12 * g + 4 * g + p // 32] = 1.0

    kstart = [96, 128, 32, 64, 0]   # pass t -> k block start
    p_idx = np.arange(128)
    bdu16 = bdu.astype(np.float16)
    in_maps = []
    for core in range(N_CORES):
        cs = core * CPC
        cstv = np.zeros((128, 321), dtype=np.float32)
        for g in range(3):
            cc = cs + p_idx // 32          # conjunct per partition
            kk = p_idx % 32
            for tpass in range(5):
                cstv[:, g * 5 + tpass] = d[g, cc, kstart[tpass] + kk]
                cstv[:, 15 + g * 5 + tpass] = e[g, cc, kstart[tpass] + kk]
        cstv[:, 30:286] = xu_in
        cstv[:, 286:302] = xn_in
        for p in range(12):
            cstv[p, 302] = np.float32(np.log(ok[p // 4, cs + p % 4]))
        cstv[:, 303:321] = bdu16.view(np.float32)
        in_maps.append({"xb": xb_in, "xbT": xbT_in, "cst": cstv})
    return ak.astype(np.float32), ok.astype(np.float32), in_maps


def _host_post(u_grid, ak, ok):
    """u_grid [R, C, GRID] f32: per-conjunct 1 - ok*conj from the cores."""
    disj = (1.0 - np.prod(u_grid, axis=1)).reshape(R, B, O, O)
    a_ = np.repeat(np.arange(O), O - 1)
    bbi = np.tile(np.arange(O - 1), O)
    bb = bbi + (bbi >= a_)
    rules = disj[:, :, a_, bb]                             # [R,B,NPERM]
    rules = np.moveaxis(rules, 0, -1).reshape(B, O, O - 1, R)
    nullary_rules = (1.0 - np.prod(1.0 - rules[..., 0], axis=(1, 2))
                     )[:, None].astype(np.float32)
    unary_rules = (1.0 - np.prod(1.0 - rules[..., 1], axis=2)
                   )[:, :, None].astype(np.float32)
    binary_rules = rules[..., 2:3].astype(np.float32)
    return nullary_rules, unary_rules, binary_rules, ak, ok


def kernel(nullary, unary, binary, and_kernel, or_kernel, temperature):
    nullary = np.asarray(nullary, dtype=np.float32)
    unary = np.asarray(unary, dtype=np.float32)
    binary = np.asarray(binary, dtype=np.float32)
    and_kernel = np.asarray(and_kernel, dtype=np.float32)
    or_kernel = np.asarray(or_kernel, dtype=np.float32)
    temperature = np.asarray(temperature, dtype=np.float32)

    ak, ok, in_maps = _host_prep(nullary, unary, binary,
                                 and_kernel, or_kernel, temperature)

    import os
    if "nc" not in _cache:
        _cache["nc"] = _build_bass()
    from concourse.bass_utils import run_bass_kernel_spmd
    res = run_bass_kernel_spmd(_cache["nc"], in_maps,
                               core_ids=list(range(N_CORES)),
                               trace=bool(os.environ.get("KERNEL_TRACE")))
    _cache["last_results"] = res
    u_grid = np.empty((R, C, GRID), dtype=np.float32)
    for i, r in enumerate(res.results):
        negt = r["uo"]                    # -t; rc = r*4 + cc_local
        for rr in range(R):
            u_grid[rr, i * CPC:(i + 1) * CPC] = (
                1.0 + negt[4 * rr:4 * rr + 4])
    return _host_post(u_grid, ak, ok)
